# revision 32
# baseline (speedup 1.0000x reference)
"""Trainium2 Bass kernel for nn_BaseTransformer (B=16, C=128, L=1024, H=8, dk=dv=32).

Sharding: pure data-parallel over batch — 8 cores x 2 batches each, no collectives.

v2 design (multi-engine softmax + fp8 DoubleRow PV):
  - QK projection + attention logits in bf16 (accuracy-critical): PE streams
    out-free rows, so logits cost is fixed at ~55us/core regardless of dtype.
  - Softmax exp is the Activation-engine bottleneck (131072 rows/core at
    1 row/cycle @1.2GHz). Split it across engines per i-block pair:
      A-pairs: ScalarE exact exp with bias=-CSHIFT, output fp8e4m3 (max 240,
               e^(11.5-6.5)=148 safe); feeds fp8 DoubleRow PV matmuls that
               process TWO key-blocks per instruction (2x PE win).
      S-pairs: DVE/Pool Schraudolph fast-exp: one tensor_scalar computing
               int16(l*A + B) whose bit pattern IS bf16(e^(l-CSHIFT)) (~3%);
               feeds bf16 PV matmuls. The constant shift cancels in the
               softmax normalization since the denominator uses the same st.
  - Softmax denominator merged into PV via [v|ones] stationary (denmerge);
    normalization mults can run on Pool (knob) to free DVE for exp work.
"""

import os
import numpy as np

B, C, L = 16, 128, 1024
DK, DV, H = 32, 32, 8
SCALE = DK ** (-0.5)
NCORES = 8
BLOC = B // NCORES  # batches per core

CSHIFT = 6.5
SCH_A = 128.0 / float(np.log(2.0))          # 184.664965
SCH_B = 16250.33 - SCH_A * CSHIFT           # bf16 Schraudolph magic (trunc)

_CACHE = {}


def _split_excess_waits(nc, mybir, cap=1):
    """This container's walrus rejects instructions carrying more than one
    sync-wait command ("Too many sync wait commands" in setupSyncWait), while
    Tile freely attaches several. Move all but `cap` waits of every
    instruction onto injected same-engine NoOps placed immediately before it
    (same block order == same engine queue order, so semantics are identical:
    all waits still complete before the instruction issues)."""
    ctr = 0
    for f in nc.m.functions:
        for blk in f.blocks:
            out = []
            changed = False
            for ins in blk.instructions:
                si = ins.sync_info
                waits = list(si.on_wait) if si and si.on_wait else []
                eng = getattr(ins, "engine", None)
                if len(waits) > cap and eng is not None:
                    for w in waits[:-cap]:
                        nop = mybir.InstNoOp(name=f"I-wsplit-{ctr}")
                        ctr += 1
                        nop.engine = eng
                        nop.sync_info = mybir.SyncInfo(on_wait=[w], on_update=[])
                        out.append(nop)
                    ins.sync_info = mybir.SyncInfo(
                        on_wait=waits[-cap:], on_update=list(si.on_update or [])
                    )
                    changed = True
                out.append(ins)
            if changed:
                blk.instructions = out
    return nc


def _sgroup_sets():
    """Which pp-groups (unit u in 0..7, pair p in 0..3, pp in 0..1) run
    Schraudolph, and on which engine. Returns dict[(u,p,pp)] -> 'D'|'P'
    (absent = Act). A pp-group is one head-pair's two i-blocks (2048 rows),
    the granularity at which the fp8-DoubleRow PV pairing applies."""
    nsp = int(os.environ.get("KSPG", "17"))
    # NOTE: gpsimd (Pool) cannot access PSUM on real hardware, so 'P' is not
    # a legal exp engine; only 'D' (DVE) is.
    seng = os.environ.get("KSENG", "D")  # rotation over S-groups
    out = {}
    k = 0
    for n in range(64):
        if (n * nsp) // 64 != ((n + 1) * nsp) // 64:
            u, r = divmod(n, 8)
            out[(u, r // 2, r % 2)] = seng[k % len(seng)]
            k += 1
    return out


def _build_nc():
    import concourse.bass as bass
    import concourse.tile as tile
    from concourse import mybir
    from contextlib import ExitStack

    f32 = mybir.dt.float32
    bf16 = mybir.dt.bfloat16
    f8e4 = mybir.dt.float8e4
    i16 = mybir.dt.int16
    nc = bass.Bass()

    x_d = nc.dram_tensor("x_sh", [BLOC, C, L], bf16, kind="ExternalInput")
    wqk_d = nc.dram_tensor("wqk", [C, 4, 128], bf16, kind="ExternalInput")
    bqk_d = nc.dram_tensor("bqk", [128, 2], f32, kind="ExternalInput")
    wv_d = nc.dram_tensor("wv", [C, 256], bf16, kind="ExternalInput")
    wo_d = nc.dram_tensor("wo", [128, 3, 128], bf16, kind="ExternalInput")
    bout_d = nc.dram_tensor("bout", [128, 1], f32, kind="ExternalInput")
    out_d = nc.dram_tensor("out_sh", [BLOC, C, L], f32, kind="ExternalOutput")

    Exp = mybir.ActivationFunctionType.Exp
    mult = mybir.AluOpType.mult
    add = mybir.AluOpType.add
    DR = mybir.MatmulPerfMode.DoubleRow

    sgroups = _sgroup_sets()
    norm_eng = os.environ.get("KNORM", "D")  # Pool can't read PSUM: D only

    with tile.TileContext(nc) as tc, ExitStack() as ctx:
        consts = ctx.enter_context(tc.tile_pool(name="consts", bufs=1))
        xp = ctx.enter_context(tc.tile_pool(name="xp", bufs=2))
        qkp = ctx.enter_context(tc.tile_pool(name="qkp", bufs=2))
        vtp = ctx.enter_context(tc.tile_pool(name="vtp", bufs=2))
        stp = ctx.enter_context(tc.tile_pool(name="stp", bufs=4))
        zfp = ctx.enter_context(tc.tile_pool(name="zfp", bufs=2))
        rbp = ctx.enter_context(tc.tile_pool(name="rbp", bufs=3))
        outp = ctx.enter_context(tc.tile_pool(name="outp", bufs=2))
        pls = ctx.enter_context(tc.tile_pool(name="pls", bufs=3, space="PSUM"))
        pacc = ctx.enter_context(tc.tile_pool(name="pacc", bufs=1, space="PSUM"))

        wqk_sb = consts.tile([C, 4, 128], bf16, name="wqk_sb")
        bqk_sb = consts.tile([128, 2], f32, name="bqk_sb")
        wv_sb = consts.tile([C, 256], bf16, name="wv_sb")
        wo_sb = consts.tile([128, 3, 128], bf16, name="wo_sb")
        bout_sb = consts.tile([128, 1], f32, name="bout_sb")
        cshift_sb = consts.tile([128, 1], f32, name="cshift_sb")
        nc.vector.memset(cshift_sb, -CSHIFT)
        # spread weight-load issue across queues so x_sh (issued first in
        # _proj, SP queue) and the projection weights all land by ~2us
        nc.scalar.dma_start(out=wqk_sb, in_=wqk_d[:, :, :])
        nc.scalar.dma_start(out=wv_sb, in_=wv_d[:, :])
        nc.gpsimd.dma_start(out=bqk_sb, in_=bqk_d[:, :])
        nc.gpsimd.dma_start(out=wo_sb, in_=wo_d[:, :, :])
        nc.gpsimd.dma_start(out=bout_sb, in_=bout_d[:, :])

        repeat = int(os.environ.get("KREPEAT", "1"))
        for _rep in range(repeat):
          S = {}

          def _init_batch(b):
            S[b] = dict(
                x_sb=xp.tile([C, L], bf16, name="x_sb"),
                qA=qkp.tile([128, L], bf16, name="qA"),
                qB=qkp.tile([128, L], bf16, name="qB"),
                kA=qkp.tile([128, L], bf16, name="kA"),
                kB=qkp.tile([128, L], bf16, name="kB"),
                zfA=zfp.tile([128, L], bf16, name="zfA"),
                zfB=zfp.tile([128, L], bf16, name="zfB"),
            )

          def _xdma(b):
            nc.sync.dma_start(out=S[b]["x_sb"], in_=x_d[b])

          def _qkchunk(b, cch):
            # one QK-projection chunk: psum -> (q bias-add | k copy) -> SBUF
            tgts = [S[b]["qA"], S[b]["qB"], S[b]["kA"], S[b]["kB"]]
            x_sb = S[b]["x_sb"]
            ps = pls.tile([128, 2, 512], f32, name="pl")
            for jh in range(2):
                sj = slice(512 * jh, 512 * jh + 512)
                nc.tensor.matmul(
                    out=ps[:, jh, :],
                    lhsT=wqk_sb[:, cch, :],
                    rhs=x_sb[:, sj],
                    start=True, stop=True,
                )
            tgt = tgts[cch].rearrange("p (a f) -> p a f", a=2)
            if cch < 2:
                nc.vector.tensor_scalar_add(
                    out=tgt, in0=ps, scalar1=bqk_sb[:, cch : cch + 1]
                )
            else:
                nc.vector.tensor_copy(out=tgt, in_=ps)

          def _vchunk(b, gq):
            # ---- V^T projection -> vtf8 (fp8, i-pair planes) + vtbf (bf16)
            # Each head h gets a 128-wide zero-padded stationary so every PV
            # matmul is a full M=128 tile at tile_position (0,0) (the only
            # placement walrus accepts for DoubleRow):
            #   even heads: [v(32) | ones(32) | zeros(64)]  -> [z|den] at
            #               psum partitions 0..63
            #   odd heads:  [zeros(64) | v(32) | ones(32)]  -> 64..127
            # dim3 order is evens-first [h0 h2 h4 h6 h1 h3 h5 h7], matching
            # the evens-first wv column permutation done host-side.
            x_sb = S[b]["x_sb"]
            if gq == 0:
                S[b]["vtf8"] = vtp.tile([128, 4, 2, 8, 128], f8e4, name="vtf8")
                S[b]["vtbf"] = vtp.tile([128, 8, 8, 128], bf16, name="vtbf")
            vtf8, vtbf = S[b]["vtf8"], S[b]["vtbf"]
            ps = pls.tile([128, 1024], f32, name="pl")
            for q in range(4):
                i = 4 * gq + q
                nc.tensor.matmul(
                    out=ps[:, 256 * q : 256 * q + 256],
                    lhsT=x_sb[:, 128 * i : 128 * i + 128],
                    rhs=wv_sb,
                    start=True, stop=True,
                )
            r8 = ps.rearrange("p (a pl hf he d) -> p a pl hf he d",
                              a=2, pl=2, hf=2, he=4, d=32)
            nc.vector.tensor_copy(
                out=vtf8[:, 2 * gq : 2 * gq + 2, :, 0:4, 0:32],
                in_=r8[:, :, :, 0])
            nc.vector.tensor_copy(
                out=vtf8[:, 2 * gq : 2 * gq + 2, :, 4:8, 64:96],
                in_=r8[:, :, :, 1])
            rb2 = ps.rearrange("p (ab hf he d) -> p ab hf he d",
                               ab=4, hf=2, he=4, d=32)
            nc.vector.tensor_copy(
                out=vtbf[:, 4 * gq : 4 * gq + 4, 0:4, 0:32],
                in_=rb2[:, :, 0])
            nc.vector.tensor_copy(
                out=vtbf[:, 4 * gq : 4 * gq + 4, 4:8, 64:96],
                in_=rb2[:, :, 1])
            if gq == 1:
                for vt in (vtf8, vtbf):
                    sl = (slice(None),) * (len(vt.shape) - 2)
                    nc.gpsimd.memset(vt[sl + (slice(0, 4), slice(32, 64))], 1.0)
                    nc.gpsimd.memset(vt[sl + (slice(0, 4), slice(64, 128))], 0.0)
                    nc.gpsimd.memset(vt[sl + (slice(4, 8), slice(0, 64))], 0.0)
                    nc.gpsimd.memset(vt[sl + (slice(4, 8), slice(96, 128))], 1.0)

          steps = []

          def _attn_unit(b, g, j):
            q_t = (S[b]["qA"], S[b]["qB"])[g]
            k_t = (S[b]["kA"], S[b]["kB"])[g]
            zf = (S[b]["zfA"], S[b]["zfB"])[g]
            unit = (b * 2 + g) * 2 + j
            sj = slice(512 * j, 512 * j + 512)
            ust = {}

            def mk(p):
                engs = [sgroups.get((unit, p, pp)) for pp in range(2)]
                st_h = {}

                def qkexp():
                    for ii in range(2):
                        i = 2 * p + ii
                        for pp in range(2):
                            seng = engs[pp]
                            pl = pls.tile([128, 2, 512], f32, name="pl")
                            for hh in range(2):
                                rr = 64 * pp + 32 * hh
                                nc.tensor.matmul(
                                    out=pl[:, hh, :],
                                    lhsT=k_t[rr : rr + 32, 128 * i : 128 * i + 128],
                                    rhs=q_t[rr : rr + 32, sj],
                                    start=True, stop=True,
                                    tile_position=(rr, 0),
                                )
                            if seng is None:
                                if ii == 0:
                                    st_h[pp] = stp.tile([128, 2, 2, 512], f8e4,
                                                        name="stf8", bufs=6)
                                nc.scalar.activation(
                                    out=st_h[pp][:, ii, :, :],
                                    in_=pl, func=Exp,
                                    bias=cshift_sb[:, 0:1],
                                )
                            else:
                                eng = nc.vector if seng == "D" else nc.gpsimd
                                stb = stp.tile([128, 2, 512], bf16,
                                               name="stb", bufs=6)
                                eng.tensor_scalar(
                                    out=stb[:, :, :].bitcast(i16),
                                    in0=pl,
                                    scalar1=SCH_A, scalar2=SCH_B,
                                    op0=mult, op1=add,
                                )
                                st_h[(ii, pp)] = stb

                def pv():
                    if "comb" not in ust:
                        ust["comb"] = pacc.tile([128, 2, 512], f32, name="comb")
                        ust["cnt"] = [0, 0]
                        ust["tot"] = [
                            sum(2 if sgroups.get((unit, q, w)) is None else 4
                                for q in range(4))
                            for w in range(2)]
                    comb = ust["comb"]

                    def flags(pp):
                        idx = ust["cnt"][pp]
                        ust["cnt"][pp] += 1
                        return dict(start=(idx == 0),
                                    stop=(idx == ust["tot"][pp] - 1))

                    for pp in range(2):
                        if engs[pp] is None:
                            for hh in range(2):
                                h = 4 * g + 2 * pp + hh
                                nc.tensor.matmul(
                                    out=comb[:, pp, :],
                                    lhsT=S[b]["vtf8"][:, p, :,
                                                      (h % 2) * 4 + h // 2, :],
                                    rhs=st_h[pp][:, :, hh, :],
                                    perf_mode=DR,
                                    skip_group_check=True,
                                    **flags(pp),
                                )
                        else:
                            for ii in range(2):
                                i = 2 * p + ii
                                for hh in range(2):
                                    h = 4 * g + 2 * pp + hh
                                    nc.tensor.matmul(
                                        out=comb[:, pp, :],
                                        lhsT=S[b]["vtbf"][:, i,
                                                          (h % 2) * 4 + h // 2, :],
                                        rhs=st_h[(ii, pp)][:, hh, :],
                                        skip_group_check=True,
                                        **flags(pp),
                                    )
                    if p == 3:
                        # normalization: z * (1/den); den replicated over
                        # 32 partitions of the comb tile. Mults split over
                        # DVE+Pool so the comb slot frees quickly (ring of 1).
                        rb = rbp.tile([128, 2, 512], f32, name="rb")
                        if os.environ.get("KRECIP", "V") == "F":
                            # custom-DVE ops hit "ISA wrong length" in this
                            # container's walrus; keep the plain reciprocal.
                            nc.vector.reciprocal_approx_fast(out=rb, in_=comb)
                        else:
                            nc.vector.reciprocal(out=rb, in_=comb)
                        for pp in range(2):
                            for hh in range(2):
                                h4 = (2 * pp + hh) % 4
                                if norm_eng == "S":
                                    meng = nc.vector if hh == 0 else nc.gpsimd
                                elif norm_eng == "P":
                                    meng = nc.gpsimd
                                else:
                                    meng = nc.vector
                                meng.tensor_tensor(
                                    out=zf[32 * h4 : 32 * h4 + 32, sj],
                                    in0=comb[64 * hh : 64 * hh + 32, pp, :],
                                    in1=rb[64 * hh + 32 : 64 * hh + 64, pp, :],
                                    op=mult,
                                )

                return qkexp, pv

            for p in range(4):
                qkexp, pv = mk(p)
                wo_half = (b, j) if (g == 1 and p == 3) else None
                steps.append((qkexp, pv, p, wo_half))

          def _wo(b, j):
            # half of the output projection for s-range j; runs as soon as
            # both head-groups' zf halves for this j are normalized.
            x_sb, zfA, zfB = (S[b][k] for k in ("x_sb", "zfA", "zfB"))
            if "o_sb" not in S[b]:
                S[b]["o_sb"] = outp.tile([128, L], f32, name="o_sb")
            o_sb = S[b]["o_sb"]
            sj = slice(512 * j, 512 * j + 512)
            po = pls.tile([128, 512], f32, name="pl")
            nc.tensor.matmul(out=po, lhsT=wo_sb[:, 0, :], rhs=zfA[:, sj],
                             start=True, stop=False)
            nc.tensor.matmul(out=po, lhsT=wo_sb[:, 1, :], rhs=zfB[:, sj],
                             start=False, stop=False)
            nc.tensor.matmul(out=po, lhsT=wo_sb[:, 2, :], rhs=x_sb[:, sj],
                             start=False, stop=True)
            nc.vector.tensor_scalar_add(out=o_sb[:, sj], in0=po,
                                        scalar1=bout_sb[:, 0:1])
            nc.sync.dma_start(out=out_d[b][:, sj], in_=o_sb[:, sj])

          # unit order j-major within a batch so both zf[:, j]-halves finish
          # early and the W_o half for j can issue mid-stream.
          for b in range(BLOC):
            _init_batch(b)
          for b in range(BLOC):
            for j in range(2):
              for g in range(2):
                _attn_unit(b, g, j)

          # projection chunks interleaved into the early attention steps so
          # the Act/DVE engines aren't idle during the projection phase
          _xdma(0)
          _xdma(1)
          for c in (0, 2):
              _qkchunk(0, c)
          _vchunk(0, 0)
          _vchunk(0, 1)
          pre_at = {0: [(_qkchunk, (0, 1))], 1: [(_qkchunk, (0, 3))],
                    2: [(_qkchunk, (1, 0))], 3: [(_qkchunk, (1, 2))],
                    4: [(_vchunk, (1, 0))], 5: [(_vchunk, (1, 1))],
                    6: [(_qkchunk, (1, 1))], 7: [(_qkchunk, (1, 3))]}

          # Software pipeline: QK+exp of step s+1 (s+2 across unit
          # boundaries, hiding the norm latency that frees the comb slot)
          # issue before PV of step s, so the PE never stalls behind a PV
          # waiting on its pair's exp.
          lag2 = int(os.environ.get("KLAG2", "1"))
          n = len(steps)
          due = [[] for _ in range(n + 3)]
          for idx, (qkexp, pv, p, wo_half) in enumerate(steps):
              slot = idx + (2 if (p == 3 and lag2) else 1)
              due[min(slot, n + 2)].append((pv, wo_half))
          for idx in range(n + 3):
              if idx < n:
                  steps[idx][0]()
                  for fn, args in pre_at.get(idx, ()):
                      fn(*args)
              for pv, wo_half in due[idx]:
                  pv()
                  if wo_half is not None:
                      _wo(*wo_half)

    _split_excess_waits(nc, mybir)
    nc.finalize()
    return nc


def get_nc():
    if "nc" not in _CACHE:
        _CACHE["nc"] = _build_nc()
    return _CACHE["nc"]


def prep_weights(w_qkv, b_qkv, w_o, b_o, w_res, b_res):
    w_qkv = np.asarray(w_qkv, np.float32)
    b_qkv = np.asarray(b_qkv, np.float32)
    w_o = np.asarray(w_o, np.float32)
    b_o = np.asarray(b_o, np.float32)
    w_res = np.asarray(w_res, np.float32)
    b_res = np.asarray(b_res, np.float32)

    d = np.arange(32)
    qrows = np.concatenate([96 * h + d for h in range(H)])        # (256,)
    krows = np.concatenate([96 * h + 32 + d for h in range(H)])
    vrows = np.concatenate([96 * h + 64 + d for h in range(H)])

    Wq = w_qkv[qrows] * SCALE                                     # (256, C)
    Wk = w_qkv[krows]
    wqk = np.stack([Wq[:128].T, Wq[128:].T, Wk[:128].T, Wk[128:].T], axis=1)
    bqk = np.stack([b_qkv[qrows[:128]], b_qkv[qrows[128:]]], axis=1) * SCALE
    # v columns evens-first to match the vt zero-padded stationary layout
    vrows_ef = np.concatenate([96 * h + 64 + d for h in (0, 2, 4, 6, 1, 3, 5, 7)])
    wv = np.ascontiguousarray(w_qkv[vrows_ef].T)                  # (C, 256)
    wo = np.stack([w_o[:, :128].T, w_o[:, 128:].T, w_res.T], axis=1)
    bv = b_qkv[vrows]
    bout = (b_o + b_res + w_o @ bv)[:, None]

    import ml_dtypes
    bf = ml_dtypes.bfloat16
    return {
        "wqk": np.ascontiguousarray(wqk, bf),
        "bqk": np.ascontiguousarray(bqk, np.float32),
        "wv": np.ascontiguousarray(wv, bf),
        "wo": np.ascontiguousarray(wo, bf),
        "bout": np.ascontiguousarray(bout, np.float32),
    }


def make_in_maps(x, weights):
    import ml_dtypes
    x = np.ascontiguousarray(np.asarray(x).astype(ml_dtypes.bfloat16))
    return [
        dict(x_sh=np.ascontiguousarray(x[BLOC * i : BLOC * i + BLOC]), **weights)
        for i in range(NCORES)
    ]


class Runner:
    """Persistent PJRT executable for the SPMD bass program (axon path).

    Mirrors concourse.bass2jax.run_bass_via_pjrt's multi-core branch, but keeps
    the jitted callable so repeated executions don't re-trace/re-compile —
    needed both for a fast kernel() and for timing loops in test.py.
    """

    def __init__(self, nc=None, donate=True):
        import jax
        import concourse.mybir as mybir
        from concourse import bass2jax
        from jax.experimental.shard_map import shard_map
        from jax.sharding import Mesh, PartitionSpec

        if nc is None:
            nc = get_nc()
        bass2jax.install_neuronx_cc_hook()

        in_names, out_names, out_avals = [], [], []
        partition_name = (
            nc.partition_id_tensor.name if nc.partition_id_tensor else None
        )
        for alloc in nc.m.functions[0].allocations:
            if not isinstance(alloc, mybir.MemoryLocationSet):
                continue
            name = alloc.memorylocations[0].name
            if alloc.kind == "ExternalInput":
                if name != partition_name:
                    in_names.append(name)
            elif alloc.kind == "ExternalOutput":
                shape = tuple(alloc.tensor_shape)
                dtype = mybir.dt.np(alloc.dtype)
                out_avals.append(jax.core.ShapedArray(shape, dtype))
                out_names.append(name)
        n_params = len(in_names)
        n_outs = len(out_avals)
        all_in_names = list(in_names) + list(out_names)
        if partition_name is not None:
            all_in_names.append(partition_name)
        self.in_names = in_names
        self.out_names = out_names
        self.out_avals = out_avals

        donate_idx = tuple(range(n_params, n_params + n_outs)) if donate else ()

        def _body(*args):
            operands = list(args)
            if partition_name is not None:
                operands.append(bass2jax.partition_id_tensor())
            outs = bass2jax._bass_exec_p.bind(
                *operands,
                out_avals=tuple(out_avals),
                in_names=tuple(all_in_names),
                out_names=tuple(out_names),
                lowering_input_output_aliases=(),
                sim_require_finite=True,
                sim_require_nnan=True,
                nc=nc,
            )
            return tuple(outs)

        devices = jax.devices()[:NCORES]
        assert len(devices) == NCORES
        mesh = Mesh(np.asarray(devices), ("core",))
        in_specs = (PartitionSpec("core"),) * (n_params + n_outs)
        out_specs = (PartitionSpec("core"),) * n_outs
        self.sharded = jax.jit(
            shard_map(_body, mesh=mesh, in_specs=in_specs, out_specs=out_specs,
                      check_rep=False),
            donate_argnums=donate_idx,
            keep_unused=True,
        )
        self.mesh = mesh

    def prep(self, in_maps):
        return [
            np.concatenate([np.asarray(m[name]) for m in in_maps], axis=0)
            for name in self.in_names
        ]

    def zeros(self):
        return [
            np.zeros((NCORES * a.shape[0], *a.shape[1:]), a.dtype)
            for a in self.out_avals
        ]

    def call_async(self, concat_in):
        return self.sharded(*concat_in, *self.zeros())

    def __call__(self, in_maps):
        outs = self.call_async(self.prep(in_maps))
        arr = np.asarray(outs[0])
        return arr.reshape(NCORES, *self.out_avals[0].shape)


def get_runner():
    if "runner" not in _CACHE:
        _CACHE["runner"] = Runner()
    return _CACHE["runner"]


def run(x, weights, **kw):
    runner = get_runner()
    per_core = runner(make_in_maps(x, weights))
    out = per_core.reshape(B, C, L)
    return out, None


def kernel(x, w_qkv, b_qkv, w_o, b_o, w_res, b_res):
    weights = prep_weights(w_qkv, b_qkv, w_o, b_o, w_res, b_res)
    out, _ = run(x, weights)
    return out
